# revision 1
# baseline (speedup 1.0000x reference)
# Trainium2 Bass kernel for nn_MultiCondLayer:
#   out[b,o,n] = (sum_k (cond[b] @ W[k].T)[o,n] + sum_k b[k,o]) * x_mask[b,0,n]
# Key algebraic reduction: sum_k Linear_k(x) == Linear(x) with W' = sum_k W[k],
# b' = sum_k b[k]  (4x FLOP reduction vs. the naive einsum over k).
#
# Sharding: data-parallel over batch B=8 across the 8 NeuronCores (one batch
# element per core); the reduced [1024,1024] weight is replicated.
# Per-core compute: [1024c,4096n] activations x [1024c,1024o] weights as
# 512 PE matmuls (128x128 lhsT, 128x512 rhs, fp32r = full-rate 1 cyc/row)
# accumulating in PSUM, evicted by a single fused DVE op: (psum+bias)*mask.
#
# Schedule (measured ~145-147us on HW, vs ~116us pure-matmul floor and
# ~105us HBM floor): x-in streams alone on the Sync HWDGE queue in per-c
# [128,1024] chunks (4KB descriptors); weights (o-halved 256KB chunks),
# bias and out-stores ride the Activation HWDGE queue; the x_mask row is
# broadcast across partitions on-chip by the PE (ones outer product) while
# it is otherwise DMA-starved at startup. Matmuls run c-outer/o4/nsub-inner
# so each weight tile feeds 2 back-to-back matmuls and 8 PSUM banks stay
# in flight; evictions+stores chase each psum group closely.

import numpy as np
from contextlib import ExitStack

import concourse.bass as bass
import concourse.mybir as mybir
import concourse.tile as tile
from concourse import bacc
from concourse.bass_utils import run_bass_kernel_spmd

P = 128
B, C, N = 8, 1024, 4096
O = 1024
NT = 512                 # matmul free dim = one fp32 PSUM bank
CO, OO, NN = C // P, O // P, N // NT
F32 = mybir.dt.float32
F32R = mybir.dt.float32r

N_CORES = 8


NSUP = 1024              # n superchunk width (4 KB DMA descriptors)
NSUPS = N // NSUP        # 4
NSUB = NSUP // NT        # 2 psum-width subchunks per superchunk


def build_module():
    nc = bacc.Bacc("TRN2", target_bir_lowering=False, debug=False,
                   num_devices=N_CORES)
    x = nc.dram_tensor("x", [C, N], F32R, kind="ExternalInput")    # cond[b]
    wt = nc.dram_tensor("wt", [C, O], F32R, kind="ExternalInput")  # (sum_k W[k]).T
    # bias pre-transposed on host to [128, OO] so the DMA is 128 contiguous
    # 32B rows instead of 1024 4-byte gather descriptors.
    bv = nc.dram_tensor("bv", [P, OO], F32, kind="ExternalInput")
    mk = nc.dram_tensor("mk", [N], F32R, kind="ExternalInput")      # x_mask[b,0]
    out = nc.dram_tensor("out", [O, N], F32, kind="ExternalOutput")

    x_r = x.ap().rearrange("(c p) n -> p c n", p=P)      # [128, CO, N]
    wt_r = wt.ap().rearrange("(c p) o -> p c o", p=P)    # [128, CO, O]

    # DMA queue split: x-in alone on the Sync HWDGE queue; weights, bias and
    # out-stores on the Activation HWDGE queue.
    with tile.TileContext(nc) as tc:
        with (
            tc.tile_pool(name="consts", bufs=1) as consts,
            tc.tile_pool(name="xs", bufs=2) as xs,
            tc.tile_pool(name="outs", bufs=16) as outs,
            tc.tile_pool(name="ps", bufs=8, space="PSUM") as psp,
        ):
            # Mask broadcast built on-chip: the 16 KB mask row lands
            # instantly, then the (cold, otherwise DMA-starved) PE
            # outer-products it with a ones column into all 128 partitions.
            # Cheaper than a 2 MiB mask DMA in the congested startup window,
            # and it warms up HAM before the main stream.
            mkrow_sb = consts.tile([1, N], F32R)
            nc.scalar.dma_start(mkrow_sb[:], mk.ap()[None, :])
            ones_sb = consts.tile([1, P], F32)
            nc.vector.memset(ones_sb[:], 1.0)
            mask_sb = consts.tile([P, N], F32)
            for n in range(NN):
                # One full rotation of the shared 8-bank psum tag; the DVE
                # copies release the slots before the first real group lands.
                mps = psp.tile([P, NT], F32, name=f"mps_{n}", tag="ps")
                nc.tensor.matmul(mps[:], ones_sb[:].bitcast(F32R),
                                 mkrow_sb[:, n * NT:(n + 1) * NT],
                                 start=True, stop=True)
                nc.vector.tensor_copy(mask_sb[:, n * NT:(n + 1) * NT], mps[:])
            # Weights in per-(o-half, c) 256 KB chunks: the first matmul is
            # gated by just w[og0,c0]+x[c0] (~0.75 MB). og0 weights are
            # interleaved 1:1 with the first superchunk's x chunks on the
            # Sync queue so the sc1 x prefetch cannot starve them; og1
            # weights ride the otherwise-idle Q10.
            OH = O // 2
            w_sb = consts.tile([P, CO, O], F32R)
            bias_sb = consts.tile([P, OO], F32)
            for c in range(CO):
                nc.scalar.dma_start(w_sb[:, c, 0:OH], wt_r[:, c, 0:OH])
            nc.scalar.dma_start(bias_sb[:], bv.ap())
            for c in range(CO):
                nc.scalar.dma_start(w_sb[:, c, OH:O], wt_r[:, c, OH:O])

            for ns in range(NSUPS):
                x_sb = xs.tile([P, CO, NSUP], F32R, name=f"x_sb_{ns}",
                               tag="x_sb")
                for c in range(CO):
                    nc.sync.dma_start(
                        x_sb[:, c, :], x_r[:, c, ns * NSUP:(ns + 1) * NSUP])
                for og in range(2):
                    # 8 psum groups = 4 o-chunks x 2 n-subchunks; each weight
                    # tile feeds 2 back-to-back matmuls (nsub pair).
                    pss = [[psp.tile([P, NT], F32, name=f"ps_{ns}_{og}_{o4}_{nsub}",
                                     tag="ps")
                            for nsub in range(NSUB)] for o4 in range(4)]
                    for c in range(CO):
                        for o4 in range(4):
                            o = og * 4 + o4
                            for nsub in range(NSUB):
                                nc.tensor.matmul(
                                    pss[o4][nsub][:],
                                    w_sb[:, c, o * P:(o + 1) * P],
                                    x_sb[:, c, nsub * NT:(nsub + 1) * NT],
                                    start=(c == 0),
                                    stop=(c == CO - 1),
                                )
                    for o4 in range(4):
                        o = og * 4 + o4
                        for nsub in range(NSUB):
                            n0 = ns * NSUP + nsub * NT
                            ot = outs.tile([P, NT], F32,
                                           name=f"ot_{ns}_{og}_{o4}_{nsub}",
                                           tag="ot")
                            nc.vector.scalar_tensor_tensor(
                                ot[:], pss[o4][nsub][:],
                                bias_sb[:, o:o + 1], mask_sb[:, n0:n0 + NT],
                                op0=mybir.AluOpType.add, op1=mybir.AluOpType.mult,
                            )
                            nc.scalar.dma_start(
                                out.ap()[o * P:(o + 1) * P, n0:n0 + NT], ot[:])
    nc.compile()
    return nc


_NC_CACHE = None


def _get_module():
    global _NC_CACHE
    if _NC_CACHE is None:
        _NC_CACHE = build_module()
    return _NC_CACHE


def _make_in_maps(cond, x_mask, W, b):
    wt = np.ascontiguousarray(W.sum(axis=0).T, dtype=np.float32)   # [C, O]
    bv = np.ascontiguousarray(
        b.sum(axis=0).reshape(OO, P).T, dtype=np.float32)          # [128, OO]
    in_maps = []
    for core in range(N_CORES):
        in_maps.append({
            "x": np.ascontiguousarray(cond[core], dtype=np.float32),
            "wt": wt,
            "bv": bv,
            "mk": np.ascontiguousarray(x_mask[core, 0], dtype=np.float32),
        })
    return in_maps


def run(cond, x_mask, W, b, trace=False, trace_cores=None):
    """Run on hardware; returns (out [B,O,N] fp32, BassKernelResults)."""
    nc = _get_module()
    in_maps = _make_in_maps(cond, x_mask, W, b)
    res = run_bass_kernel_spmd(
        nc, in_maps, core_ids=list(range(N_CORES)),
        trace=trace, trace_cores=trace_cores,
    )
    out = np.stack([res.results[i]["out"] for i in range(N_CORES)], axis=0)
    return out, res


def kernel(cond, x_mask, W, b):
    out, _ = run(cond, x_mask, W, b)
    return out



# revision 2
# speedup vs baseline: 1.1066x; 1.1066x over previous
# Trainium2 Bass kernel for nn_MultiCondLayer:
#   out[b,o,n] = (sum_k (cond[b] @ W[k].T)[o,n] + sum_k b[k,o]) * x_mask[b,0,n]
# Algebraic reduction: sum_k Linear_k(x) == Linear(x) with W' = sum_k W[k],
# b' = sum_k b[k]  (4x FLOP reduction vs. the naive einsum over k).
#
# Sharding: data-parallel over batch B=8 across the 8 NeuronCores (one batch
# element per core); the reduced [1024,1024] weight is replicated.
#
# Precision: all operands are cast to bf16 on the host (x, W', mask) and the
# output is stored bf16 and upcast on the host. PSUM accumulation stays fp32.
# The PE streams bf16 at the same 1 col/cycle as fp32r, so this does not
# change the ~110us matmul floor, but it (a) halves HBM traffic 38->19 MB
# per core, (b) enables FWL so LDWEIGHTS (~330ns in fp32) fully hides, and
# (c) halves the startup ramp and store tail. End-to-end rel err ~2e-3,
# well under the 2e-2 gate.
#
# Schedule: at body start the PE warms the HAM clock gate with 8 dummy
# matmuls on memset data (no DMA dependency), then broadcasts the mask row
# across partitions via ones-outer-product (real work that keeps warming).
# DMA queues: mask row on the otherwise-idle gpsimd SWDGE queue; x window
# chunks on the sync HWDGE queue; weights (o-halved), bias, and out-stores
# on the scalar HWDGE queue. Main stream: for each 1024-wide n-window and
# each o-tile, one serial c-chain of 8x(LDW + 2 matmuls) accumulating a
# 2-bank psum pair, evicted by fused DVE (psum+bias)*mask into a bf16
# [128,1024] out tile and stored. Only 2 psum banks per chain are in
# flight, so evictions stagger and bank reuse has ~4 chains of slack.

import numpy as np

import ml_dtypes

import concourse.bass as bass
import concourse.mybir as mybir
import concourse.tile as tile
from concourse import bacc
from concourse.bass_utils import run_bass_kernel_spmd

P = 128
B, C, N = 8, 1024, 4096
O = 1024
NT = 512                 # matmul free dim = one fp32 PSUM bank
CO, OO = C // P, O // P
NWIN = 1024              # n window = psum pair width = out tile width
NWINS = N // NWIN
F32 = mybir.dt.float32
BF16 = mybir.dt.bfloat16

N_CORES = 8
N_WARM = 8               # dummy matmuls to warm the HAM clock gate


def build_module():
    nc = bacc.Bacc("TRN2", target_bir_lowering=False, debug=False,
                   num_devices=N_CORES)
    x = nc.dram_tensor("x", [C, N], BF16, kind="ExternalInput")    # cond[b]
    wt = nc.dram_tensor("wt", [C, O], BF16, kind="ExternalInput")  # (sum_k W[k]).T
    # bias pre-transposed on host to [128, OO]: 128 contiguous rows.
    bv = nc.dram_tensor("bv", [P, OO], F32, kind="ExternalInput")
    mk = nc.dram_tensor("mk", [N], BF16, kind="ExternalInput")     # x_mask[b,0]
    out = nc.dram_tensor("out", [O, N], BF16, kind="ExternalOutput")

    x_r = x.ap().rearrange("(c p) n -> p c n", p=P)      # [128, CO, N]
    wt_r = wt.ap().rearrange("(c p) o -> p c o", p=P)    # [128, CO, O]

    with tile.TileContext(nc) as tc:
        with (
            tc.tile_pool(name="consts", bufs=1) as consts,
            tc.tile_pool(name="outs", bufs=6) as outs,
            tc.tile_pool(name="ps", bufs=8, space="PSUM") as psp,
        ):
            # --- DMA issue order matters: small things first per queue. ---
            # gpsimd (SWDGE, otherwise idle): the 8KB mask row.
            mkrow_sb = consts.tile([1, N], BF16)
            nc.gpsimd.dma_start(mkrow_sb[:], mk.ap()[None, :])
            # scalar HWDGE: o-lower-half weights per c, bias, o-upper half.
            OH = O // 2
            w_sb = consts.tile([P, CO, O], BF16)
            bias_sb = consts.tile([P, OO], F32)
            for c in range(CO):
                nc.scalar.dma_start(w_sb[:, c, 0:OH], wt_r[:, c, 0:OH])
            nc.scalar.dma_start(bias_sb[:], bv.ap())
            for c in range(CO):
                nc.scalar.dma_start(w_sb[:, c, OH:O], wt_r[:, c, OH:O])
            # sync HWDGE: x in per-(window, c) 256KB chunks, fully resident.
            x_sb = consts.tile([P, CO, N], BF16)
            for ns in range(NWINS):
                n0 = ns * NWIN
                for c in range(CO):
                    nc.sync.dma_start(x_sb[:, c, n0:n0 + NWIN],
                                      x_r[:, c, n0:n0 + NWIN])

            # --- PE warmup: no-DMA dummy matmuls release the HAM throttle
            # (cold 1.2GHz -> warm 2.4GHz needs ~3.4us of sustained busy)
            # while the first real chunks are still in flight. ---
            scratch = consts.tile([P, NT], BF16)
            nc.vector.memset(scratch[:], 0.0)
            ones_sb = consts.tile([1, P], BF16)
            nc.vector.memset(ones_sb[:], 1.0)
            for i in range(N_WARM):
                wps = psp.tile([P, NT], F32, name=f"warm_{i}", tag="ps")
                nc.tensor.matmul(wps[:], scratch[:, 0:P], scratch[:],
                                 start=True, stop=True)

            # --- Mask broadcast on-chip: ones[128,1] (x) mkrow[1,N] via PE
            # (keeps warming; avoids a 128x replicated mask DMA). ---
            mask_sb = consts.tile([P, N], BF16)
            for n in range(N // NT):
                mps = psp.tile([P, NT], F32, name=f"mps_{n}", tag="ps")
                nc.tensor.matmul(mps[:], ones_sb[:],
                                 mkrow_sb[:, n * NT:(n + 1) * NT],
                                 start=True, stop=True)
                nc.vector.tensor_copy(mask_sb[:, n * NT:(n + 1) * NT], mps[:])

            # --- Main stream: 4 windows x 8 o-tiles x (8c x 2 matmuls). ---
            for ns in range(NWINS):
                n0 = ns * NWIN
                for o in range(OO):
                    ps0 = psp.tile([P, NT], F32, name=f"ps_{ns}_{o}_0",
                                   tag="ps")
                    ps1 = psp.tile([P, NT], F32, name=f"ps_{ns}_{o}_1",
                                   tag="ps")
                    for c in range(CO):
                        w_ap = w_sb[:, c, o * P:(o + 1) * P]
                        nc.tensor.matmul(ps0[:], w_ap,
                                         x_sb[:, c, n0:n0 + NT],
                                         start=(c == 0), stop=(c == CO - 1))
                        nc.tensor.matmul(ps1[:], w_ap,
                                         x_sb[:, c, n0 + NT:n0 + NWIN],
                                         start=(c == 0), stop=(c == CO - 1))
                    ot = outs.tile([P, NWIN], BF16, name=f"ot_{ns}_{o}",
                                   tag="ot")
                    nc.vector.scalar_tensor_tensor(
                        ot[:, 0:NT], ps0[:], bias_sb[:, o:o + 1],
                        mask_sb[:, n0:n0 + NT],
                        op0=mybir.AluOpType.add, op1=mybir.AluOpType.mult)
                    nc.vector.scalar_tensor_tensor(
                        ot[:, NT:NWIN], ps1[:], bias_sb[:, o:o + 1],
                        mask_sb[:, n0 + NT:n0 + NWIN],
                        op0=mybir.AluOpType.add, op1=mybir.AluOpType.mult)
                    nc.scalar.dma_start(
                        out.ap()[o * P:(o + 1) * P, n0:n0 + NWIN], ot[:])
    nc.compile()
    return nc


_NC_CACHE = None


def _get_module():
    global _NC_CACHE
    if _NC_CACHE is None:
        _NC_CACHE = build_module()
    return _NC_CACHE


def _make_in_maps(cond, x_mask, W, b):
    bf16 = ml_dtypes.bfloat16
    wt = np.ascontiguousarray(
        W.astype(np.float32).sum(axis=0).T.astype(bf16))           # [C, O]
    bv = np.ascontiguousarray(
        b.astype(np.float32).sum(axis=0).reshape(OO, P).T,
        dtype=np.float32)                                          # [128, OO]
    in_maps = []
    for core in range(N_CORES):
        in_maps.append({
            "x": np.ascontiguousarray(cond[core].astype(bf16)),
            "wt": wt,
            "bv": bv,
            "mk": np.ascontiguousarray(x_mask[core, 0].astype(bf16)),
        })
    return in_maps


def run(cond, x_mask, W, b, trace=False, trace_cores=None):
    """Run on hardware; returns (out [B,O,N] fp32, BassKernelResults)."""
    nc = _get_module()
    in_maps = _make_in_maps(cond, x_mask, W, b)
    res = run_bass_kernel_spmd(
        nc, in_maps, core_ids=list(range(N_CORES)),
        trace=trace, trace_cores=trace_cores,
    )
    out = np.stack(
        [res.results[i]["out"].astype(np.float32) for i in range(N_CORES)],
        axis=0)
    return out, res


def kernel(cond, x_mask, W, b):
    out, _ = run(cond, x_mask, W, b)
    return out


# revision 5
# speedup vs baseline: 1.1167x; 1.0092x over previous
# Trainium2 Bass kernel for nn_MultiCondLayer:
#   out[b,o,n] = (sum_k (cond[b] @ W[k].T)[o,n] + sum_k b[k,o]) * x_mask[b,0,n]
# Algebraic reduction: sum_k Linear_k(x) == Linear(x) with W' = sum_k W[k],
# b' = sum_k b[k]  (4x FLOP reduction vs. the naive einsum over k).
#
# Sharding: data-parallel over batch B=8 across the 8 NeuronCores (one batch
# element per core); the reduced [1024,1024] weight is replicated.
#
# Precision: all operands are cast to bf16 on the host (x, W', mask) and the
# output is stored bf16 and upcast on the host. PSUM accumulation stays fp32.
# The PE streams bf16 at the same 1 col/cycle as fp32r, so this does not
# change the ~110us matmul floor, but it (a) halves HBM traffic 38->19 MB
# per core, (b) enables FWL so LDWEIGHTS (~330ns in fp32) fully hides, and
# (c) halves the startup ramp and store tail. End-to-end rel err ~2e-3,
# well under the 2e-2 gate.
#
# Schedule: at body start the PE warms the HAM clock gate with 8 dummy
# matmuls on memset data (no DMA dependency), then broadcasts the mask row
# across partitions via ones-outer-product (real work that keeps warming).
# DMA queues: mask row on the otherwise-idle gpsimd SWDGE queue; x window
# chunks on the sync HWDGE queue; weights (o-halved), bias, and out-stores
# on the scalar HWDGE queue. Main stream: for each 1024-wide n-window and
# each o-tile, one serial c-chain of 8x(LDW + 2 matmuls) accumulating a
# 2-bank psum pair, evicted by fused DVE (psum+bias)*mask into a bf16
# [128,1024] out tile and stored. Only 2 psum banks per chain are in
# flight, so evictions stagger and bank reuse has ~4 chains of slack.

import numpy as np

import ml_dtypes

import concourse.bass as bass
import concourse.mybir as mybir
import concourse.tile as tile
from concourse import bacc
from concourse.bass_utils import run_bass_kernel_spmd

P = 128
B, C, N = 8, 1024, 4096
O = 1024
NT = 512                 # matmul free dim = one fp32 PSUM bank
CO, OO = C // P, O // P
# n-window plan: narrow first window so the startup-critical x DMA is only
# 1MB (PE ramp covers it); narrow last window so the final evict+store tail
# is one bank / 128KB.
WINDOWS = [512, 1024, 1024, 1024, 512]
F32 = mybir.dt.float32
BF16 = mybir.dt.bfloat16

N_CORES = 8
N_WARM = 5               # dummy matmuls to warm the HAM clock gate


def build_module():
    nc = bacc.Bacc("TRN2", target_bir_lowering=False, debug=False,
                   num_devices=N_CORES)
    x = nc.dram_tensor("x", [C, N], BF16, kind="ExternalInput")    # cond[b]
    wt = nc.dram_tensor("wt", [C, O], BF16, kind="ExternalInput")  # (sum_k W[k]).T
    # bias pre-transposed on host to [128, OO]: 128 contiguous rows.
    bv = nc.dram_tensor("bv", [P, OO], F32, kind="ExternalInput")
    mk = nc.dram_tensor("mk", [N], BF16, kind="ExternalInput")     # x_mask[b,0]
    out = nc.dram_tensor("out", [O, N], BF16, kind="ExternalOutput")

    x_r = x.ap().rearrange("(c p) n -> p c n", p=P)      # [128, CO, N]
    wt_r = wt.ap().rearrange("(c p) o -> p c o", p=P)    # [128, CO, O]

    with tile.TileContext(nc) as tc:
        with (
            tc.tile_pool(name="consts", bufs=1) as consts,
            tc.tile_pool(name="outs", bufs=6) as outs,
            tc.tile_pool(name="ps", bufs=8, space="PSUM") as psp,
        ):
            # --- DMA issue order matters: small things first per queue. ---
            # gpsimd (SWDGE, otherwise idle): the 8KB mask row + 4KB bias.
            mkrow_sb = consts.tile([1, N], BF16)
            nc.gpsimd.dma_start(mkrow_sb[:], mk.ap()[None, :])
            bias_sb = consts.tile([P, OO], F32)
            nc.gpsimd.dma_start(bias_sb[:], bv.ap())
            # scalar HWDGE: o-lower-half weights per c, then upper half
            # (chains walk o serially, so half1 has ~7us of slack).
            OH = O // 2
            w_sb = consts.tile([P, CO, O], BF16)
            for c in range(CO):
                nc.scalar.dma_start(w_sb[:, c, 0:OH], wt_r[:, c, 0:OH])
            for c in range(CO):
                nc.scalar.dma_start(w_sb[:, c, OH:O], wt_r[:, c, OH:O])
            # sync HWDGE: x in per-(window, c) chunks, fully resident.
            x_sb = consts.tile([P, CO, N], BF16)
            n0 = 0
            for nw in WINDOWS:
                for c in range(CO):
                    nc.sync.dma_start(x_sb[:, c, n0:n0 + nw],
                                      x_r[:, c, n0:n0 + nw])
                n0 += nw

            # --- PE warmup: no-DMA dummy matmuls release the HAM throttle
            # (cold 1.2GHz -> warm 2.4GHz needs ~3.4us of sustained busy)
            # while the first real chunks are still in flight. ---
            scratch = consts.tile([P, NT], BF16)
            nc.vector.memset(scratch[:], 0.0)
            ones_sb = consts.tile([1, P], BF16)
            nc.vector.memset(ones_sb[:], 1.0)
            for i in range(N_WARM):
                wps = psp.tile([P, NT], F32, name=f"warm_{i}", tag="ps")
                nc.tensor.matmul(wps[:], scratch[:, 0:P], scratch[:],
                                 start=True, stop=True)

            # --- Mask broadcast on-chip: ones[128,1] (x) mkrow[1,N] via PE
            # (keeps warming; avoids a 128x replicated mask DMA). ---
            mask_sb = consts.tile([P, N], BF16)
            for n in range(N // NT):
                mps = psp.tile([P, NT], F32, name=f"mps_{n}", tag="ps")
                nc.tensor.matmul(mps[:], ones_sb[:],
                                 mkrow_sb[:, n * NT:(n + 1) * NT],
                                 start=True, stop=True)
                nc.vector.tensor_copy(mask_sb[:, n * NT:(n + 1) * NT], mps[:])

            # --- Main stream: per n-window, 8 serial o-chains of 8 c-steps;
            # 1024-wide windows use a 2-bank psum pair per chain, 512-wide
            # use a single bank. Only 1-2 banks in flight per chain, so
            # evictions stagger and bank reuse has ~4 chains of slack. ---
            n0 = 0
            for ns, nw in enumerate(WINDOWS):
                nsub = nw // NT
                for o in range(OO):
                    pss = [psp.tile([P, NT], F32, name=f"ps_{ns}_{o}_{j}",
                                    tag="ps") for j in range(nsub)]
                    for c in range(CO):
                        w_ap = w_sb[:, c, o * P:(o + 1) * P]
                        for j in range(nsub):
                            nj = n0 + j * NT
                            nc.tensor.matmul(pss[j][:], w_ap,
                                             x_sb[:, c, nj:nj + NT],
                                             start=(c == 0),
                                             stop=(c == CO - 1))
                    ot = outs.tile([P, nw], BF16, name=f"ot_{ns}_{o}",
                                   tag=f"ot{nw}")
                    for j in range(nsub):
                        nj = n0 + j * NT
                        nc.vector.scalar_tensor_tensor(
                            ot[:, j * NT:(j + 1) * NT], pss[j][:],
                            bias_sb[:, o:o + 1], mask_sb[:, nj:nj + NT],
                            op0=mybir.AluOpType.add,
                            op1=mybir.AluOpType.mult)
                    nc.scalar.dma_start(
                        out.ap()[o * P:(o + 1) * P, n0:n0 + nw], ot[:])
                n0 += nw
    nc.compile()
    return nc


_NC_CACHE = None


def _get_module():
    global _NC_CACHE
    if _NC_CACHE is None:
        _NC_CACHE = build_module()
    return _NC_CACHE


def _make_in_maps(cond, x_mask, W, b):
    bf16 = ml_dtypes.bfloat16
    wt = np.ascontiguousarray(
        W.astype(np.float32).sum(axis=0).T.astype(bf16))           # [C, O]
    bv = np.ascontiguousarray(
        b.astype(np.float32).sum(axis=0).reshape(OO, P).T,
        dtype=np.float32)                                          # [128, OO]
    in_maps = []
    for core in range(N_CORES):
        in_maps.append({
            "x": np.ascontiguousarray(cond[core].astype(bf16)),
            "wt": wt,
            "bv": bv,
            "mk": np.ascontiguousarray(x_mask[core, 0].astype(bf16)),
        })
    return in_maps


def run(cond, x_mask, W, b, trace=False, trace_cores=None):
    """Run on hardware; returns (out [B,O,N] fp32, BassKernelResults)."""
    nc = _get_module()
    in_maps = _make_in_maps(cond, x_mask, W, b)
    res = run_bass_kernel_spmd(
        nc, in_maps, core_ids=list(range(N_CORES)),
        trace=trace, trace_cores=trace_cores,
    )
    out = np.stack(
        [res.results[i]["out"].astype(np.float32) for i in range(N_CORES)],
        axis=0)
    return out, res


def kernel(cond, x_mask, W, b):
    out, _ = run(cond, x_mask, W, b)
    return out
